# revision 26
# baseline (speedup 1.0000x reference)
"""Trainium2 Bass kernel v7 for nn_AttenuationToRainRate (dense_mlp).

Data-parallel over 8 NeuronCores, BL = 32768 samples/core, processed in
32 "pairs" of 1024 samples (2 PSUM banks wide).

Math per sample (channels c on partitions, samples along free dim):
  style: h2 = relu(mw2 relu(mw1 md + mb1) + mb2)
  layer L: u = (SW_L h2 + sb_L) .* yc_L  +  (BW_L (h2 .* d_L) + bb_L*d_L)
           z_L = lrelu(u)            [scaled-denominator trick: no division]
  d_L = sqrt(sum_c yc_L^2 / 127) broadcast via ones-matmul (L>=1)
  L0 fold: yc_0 = w1''*x is rank-1, so w1c folds into SW_0 on the host and
           m1_0 = (SW0'' h2 + sb0'') .* broadcast(x); d_0 = k|x| row.
           Both broadcasts are stride-0-partition DMA reads from DRAM.
  out = lrelu(w5 z_3 + b5*d_3) / d_3   (division on host)

v7 engine assignment per pair (1024 cols):
  PE   : 33 matmuls (512-col each); SW/BW/ones in bf16, trunk in f32r
  Act  : h1/h2 relu, ycS sink x3, d sqrt x3, out prelu          (9 ops)
  DVE  : m1 = (scP+sb).*ycS (stt),  z = lrelu(m1+biP) (custom)  (8 ops)
  Pool : sq = ycS^2 x3, g = h2.*d x4 -- all-bf16 (gpsimd is sw-
         throughput-bound; bf16 halves its byte traffic)        (7 ops)
  DMA  : xB/d0 row broadcasts (DRAM stride-0 partition APs)
"""

import os
import sys

import numpy as np

for p in ("/opt/trn_rl_repo", "/root/.axon_site/_ro/trn_rl_repo"):
    if os.path.isdir(p) and p not in sys.path:
        sys.path.insert(0, p)


import concourse.bass as bass
import concourse.bacc as bacc
import concourse.mybir as mybir
from concourse.bass import AP
from concourse.tile import TileContext
from concourse import bass_utils

# ---- inline custom DVE op registration (kernel.py must be self-contained) --
import concourse.dve_ops as dve_ops
from concourse.dve_ops import DveOp
from concourse.dve_spec import Spec, Src0, Src1, C0, maxx, lower, _has_src1
from concourse.dve_uop import DveOpSpec
from concourse.dve_table_gen import dve_ver_for


def _register(name, spec):
    if name in dve_ops._SUB_OPCODE_FOR_NAME:
        for op in dve_ops.OPS:
            if op.name == name:
                return op
    row = max(dve_ops._SUB_OPCODE_FOR_NAME.values()) + 1
    assert row < 0x20, "custom DVE opcode row overflow"
    dve_ops._SUB_OPCODE_FOR_NAME[name] = row
    shas = {}
    for ver in ("v3", "v4"):
        try:
            tmp = DveOpSpec(name=name, opcode=row, uops=lower(spec, ver=ver),
                            rd1_en=_has_src1(spec))
            shas[ver] = tmp.sha(ver)
        except Exception:
            pass
    op = DveOp(name, spec, subdim=False, uops_sha=shas)
    dve_ops.OPS.append(op)
    dve_ops.CUSTOM_DVE_SPECS[name] = spec
    return op


def _lrelu_add_spec():
    u = Src0 + Src1
    return Spec(
        body=maxx(u, u * C0),
        reference=lambda in0, in1, s0, s1: np.maximum(in0 + in1,
                                                      (in0 + in1) * s0),
    )


LRELU_ADD_ANT = _register("LRELU_ADD_ANT", _lrelu_add_spec())


class custom_ops:
    LRELU_ADD_ANT = LRELU_ADD_ANT

B = 262144
MF = 16
C = 128
NCORES = 8
BL = B // NCORES            # 32768 samples per core
PAIR = 1024                 # samples per pair (2 PSUM banks)
NPAIR = BL // PAIR          # 32
F32R = mybir.dt.float32r
F32 = mybir.dt.float32
BF16 = mybir.dt.bfloat16
AF = mybir.ActivationFunctionType
ALU = mybir.AluOpType

# wpack column layout (f32r part)
_O_MW1 = 0        # [32 rows, 128] packed block-diag mw1 (2 chunks)
_O_MW2 = 128      # [0:64 rows, 128] mw2 ; [64:128 rows] duplicate
_O_WC = 256       # 3 x [128,128] centered trunk w2..w4
_O_W1 = 640       # [1 row, 128] w1'' = sign(w1c)*k (general path only)
_O_W5 = 768       # [128, 1] w5
NW = 769

# wpack16 column layout (bf16 part)
_H_SW = 0         # 4 x [128,128] scale style weights (L0 folded w/ w1c)
_H_BW = 512       # 4 x [128,128] bias style weights
_H_ONES = 1024    # [128,128] ones
_H_BROW = 1152    # [1 row, 4*128] bct rows (trunk bias, emit_tb)
_H_BBROW = 1664   # [1 row, 4*128] bb rows (style bias, emit_sb)
_H_B5 = 2176      # [1 row, 1] b5
NW16 = 2177

# bpack [128, 8] f32: cols = mb1packed, mb2, sb0..sb3, b1c
_BP_MB1, _BP_MB2, _BP_SB0, _BP_B1C = 0, 1, 2, 6
NBP = 7


def _build(emit_style_bias, emit_trunk_bias, b5_val, reps=1):
    nc = bacc.Bacc("TRN2", target_bir_lowering=False, debug=False)

    d_x = nc.dram_tensor("xt", [1, BL], F32R, kind="ExternalInput")
    d_md = nc.dram_tensor("mdt", [32, BL // 2], F32R, kind="ExternalInput")
    d_wp = nc.dram_tensor("wpack", [C, NW], F32R, kind="ExternalInput")
    d_wp16 = nc.dram_tensor("wpack16", [C, NW16], BF16, kind="ExternalInput")
    d_bp = nc.dram_tensor("bpack", [C, NBP], F32, kind="ExternalInput")
    d_xa = nc.dram_tensor("xat", [1, BL], BF16, kind="ExternalInput")
    d_x16 = nc.dram_tensor("xt16", [1, BL], BF16, kind="ExternalInput")
    d_out = nc.dram_tensor("out", [BL // 512, 512], F32, kind="ExternalOutput")
    d_den = nc.dram_tensor("den", [BL // 512, 512], BF16,
                           kind="ExternalOutput")

    from contextlib import ExitStack
    with TileContext(nc) as tc, ExitStack() as es:
        wp = es.enter_context(tc.tile_pool(name="wp", bufs=1))
        iop = es.enter_context(tc.tile_pool(name="iop", bufs=2))
        ewp = es.enter_context(tc.tile_pool(name="ewp", bufs=2))
        psp = es.enter_context(tc.tile_pool(name="psp", bufs=1, space="PSUM"))

        t_wp = wp.tile([C, NW], F32R)
        nc.sync.dma_start(t_wp[:], d_wp[:])
        t_wp16 = wp.tile([C, NW16], BF16)
        nc.sync.dma_start(t_wp16[:], d_wp16[:])
        t_bp = wp.tile([C, NBP], F32)
        nc.sync.dma_start(t_bp[:], d_bp[:])

        t_mw1 = t_wp[0:32, _O_MW1:_O_MW1 + 128]
        t_mw2a = t_wp[0:64, _O_MW2:_O_MW2 + 128]
        t_mw2b = t_wp[64:128, _O_MW2:_O_MW2 + 128]
        t_wc = [t_wp[:, _O_WC + i * C:_O_WC + (i + 1) * C] for i in range(3)]
        t_w1 = t_wp[0:1, _O_W1:_O_W1 + C]
        t_w5 = t_wp[:, _O_W5:_O_W5 + 1]
        t_sw = [t_wp16[:, _H_SW + L * C:_H_SW + (L + 1) * C]
                for L in range(4)]
        t_bw = [t_wp16[:, _H_BW + L * C:_H_BW + (L + 1) * C]
                for L in range(4)]
        t_ones = t_wp16[:, _H_ONES:_H_ONES + C]
        t_brow = [t_wp16[0:1, _H_BROW + L * C:_H_BROW + (L + 1) * C]
                  for L in range(4)]
        t_bbrow = [t_wp16[0:1, _H_BBROW + L * C:_H_BBROW + (L + 1) * C]
                   for L in range(4)]
        t_b5 = t_wp16[0:1, _H_B5:_H_B5 + 1]
        t_mb1 = t_bp[:, _BP_MB1:_BP_MB1 + 1]
        t_mb2 = t_bp[:, _BP_MB2:_BP_MB2 + 1]
        t_sb = [t_bp[:, _BP_SB0 + L:_BP_SB0 + L + 1] for L in range(4)]
        t_b1c = t_bp[:, _BP_B1C:_BP_B1C + 1]

        rep_cm = tc.For_i(0, reps, 1) if reps > 1 else None
        if rep_cm is not None:
            es.enter_context(rep_cm)

        state = [dict() for _ in range(NPAIR)]

        def _dram_bcast(dst_ap, dram_slice):
            # partition-broadcast from DRAM: re-read the same row into all
            # 128 partitions (stride-0 partition dim is legal on the DRAM
            # side only)
            src = AP(dram_slice.tensor, dram_slice.offset,
                     [[0, C], [1, PAIR]])
            nc.sync.dma_start(dst_ap, src)

        def pre_r0(p):
            st = state[p]
            t_md = iop.tile([32, 512], F32R, tag="md", bufs=4, name="t_md")
            nc.sync.dma_start(t_md[:], d_md[:, p * 512:(p + 1) * 512])
            st["md"] = t_md
            if emit_trunk_bias:
                t_x = iop.tile([1, PAIR], F32R, tag="x", bufs=2, name="t_x")
                nc.sync.dma_start(t_x[:], d_x[:, p * PAIR:(p + 1) * PAIR])
                st["x"] = t_x

        def head_r0(p):
            st = state[p]
            h1P = psp.tile([C, 512], F32, tag="S", bufs=2, name="h1P")
            nc.tensor.matmul(h1P[:], t_mw1, st.pop("md")[:],
                             start=True, stop=True)
            st["h1P"] = h1P
            if not emit_trunk_bias:
                # broadcast x and d0 = k|x| rows while the head computes
                xB = ewp.tile([C, PAIR], BF16, tag="xB", bufs=4, name="xB")
                _dram_bcast(xB[:], d_x16[0:1, p * PAIR:(p + 1) * PAIR])
                st["xB"] = xB
                dN = ewp.tile([C, PAIR], BF16, tag="dS", bufs=5, name="dN0")
                _dram_bcast(dN[:], d_xa[0:1, p * PAIR:(p + 1) * PAIR])
                st["dN"] = dN

        def head_r1(p):
            st = state[p]
            h1S = ewp.tile([C, 512], F32R, tag="h1S", bufs=2, name="h1S")
            nc.scalar.activation(h1S[:], st.pop("h1P")[:], AF.Relu,
                                 bias=t_mb1)
            st["h1S"] = h1S

        def head_r2(p):
            st = state[p]
            h1S = st.pop("h1S")
            h2P = psp.tile([C, PAIR], F32, tag="A", bufs=3, name="h2P")
            nc.tensor.matmul(h2P[:, 0:512], t_mw2a, h1S[0:64, :],
                             start=True, stop=True)
            nc.tensor.matmul(h2P[:, 512:PAIR], t_mw2b, h1S[64:128, :],
                             start=True, stop=True)
            st["h2P"] = h2P

        def head_r3(p):
            st = state[p]
            h2S = ewp.tile([C, PAIR], BF16, tag="h2S", bufs=6, name="h2S")
            nc.scalar.activation(h2S[:], st.pop("h2P")[:], AF.Relu,
                                 bias=t_mb2)
            st["h2S"] = h2S

        def lay_r0(p, L):
            st = state[p]
            if L == 0:
                if not emit_trunk_bias:
                    return
                ycP = psp.tile([C, PAIR], F32, tag="A", bufs=3, name="ycP")
                t_x = st.pop("x")
                nc.tensor.matmul(ycP[:, 0:512], t_w1, t_x[:, 0:512],
                                 start=True, stop=True)
                nc.tensor.matmul(ycP[:, 512:PAIR], t_w1, t_x[:, 512:PAIR],
                                 start=True, stop=True)
                st["ycP"] = ycP
                return
            ycP = psp.tile([C, PAIR], F32, tag="A", bufs=3, name="ycP")
            zS, dSp = st.pop("zS"), st["dS"]
            stp = not emit_trunk_bias
            nc.tensor.matmul(ycP[:, 0:512], t_wc[L - 1], zS[:, 0:512],
                             start=True, stop=stp)
            nc.tensor.matmul(ycP[:, 512:PAIR], t_wc[L - 1],
                             zS[:, 512:PAIR], start=True, stop=stp)
            if emit_trunk_bias:
                nc.tensor.matmul(ycP[:, 0:512], t_brow[L],
                                 dSp[0:1, 0:512], start=False, stop=True)
                nc.tensor.matmul(ycP[:, 512:PAIR], t_brow[L],
                                 dSp[0:1, 512:PAIR],
                                 start=False, stop=True)
            st["ycP"] = ycP

        def lay_r1(p, L):
            st = state[p]
            if L == 0 and not emit_trunk_bias:
                return
            # fast PSUM sink: ycS frees ycP's banks quickly (PSUM rotation
            # is the scarce resource); squares then run off SBUF on Pool.
            # L1 sinks on Act, L2/L3 on DVE (engine balance).
            ycS = ewp.tile([C, PAIR], BF16, tag="ycS", bufs=5, name="ycS")
            if L == 0 and emit_trunk_bias:
                nc.scalar.activation(ycS[:], st.pop("ycP")[:], AF.Identity,
                                     bias=t_b1c)
            elif L >= 2:
                nc.vector.tensor_copy(ycS[:], st.pop("ycP")[:])
            else:
                nc.scalar.activation(ycS[:], st.pop("ycP")[:], AF.Identity)
            st["ycS"] = ycS

        def lay_r2(p, L):
            st = state[p]
            if L == 0:
                if emit_trunk_bias:
                    dN = ewp.tile([C, PAIR], BF16, tag="dS", bufs=5,
                                  name="dN")
                    nc.scalar.activation(dN[:], st["ycS"][:], AF.Abs)
                    st["dN"] = dN
            else:
                ycS = st["ycS"]
                sqS = ewp.tile([C, PAIR], BF16, tag="sq", bufs=5, name="sqS")
                nc.gpsimd.tensor_mul(sqS[:], ycS[:], ycS[:])
                st["sqS"] = sqS

        def lay_r3(p, L):
            st = state[p]
            if L > 0:
                sqS = st.pop("sqS")
                vP = psp.tile([C, PAIR], F32, tag="A", bufs=3, name="vP")
                nc.tensor.matmul(vP[:, 0:512], t_ones, sqS[:, 0:512],
                                 start=True, stop=True)
                nc.tensor.matmul(vP[:, 512:PAIR], t_ones, sqS[:, 512:PAIR],
                                 start=True, stop=True)
                st["vP"] = vP

        def lay_r4(p, L):
            st = state[p]
            if L > 0:
                dN = ewp.tile([C, PAIR], BF16, tag="dS", bufs=5, name="dN")
                nc.scalar.activation(dN[:], st.pop("vP")[:], AF.Sqrt,
                                     scale=1.0 / (C - 1))
                st["dN"] = dN

        def lay_r5(p, L):
            st = state[p]
            dN = st["dN"]
            # scbi: merged scale/bias PSUM bank, opened as late as possible
            # (PSUM slot time is scarce). scP matmul opens the accumulation
            # (start, no stop); the DVE m1 RMW rewrites the values
            # (has_written bits survive); the BW matmul then accumulates
            # the bias path on top.
            scbi = psp.tile([C, PAIR], F32, tag="A", bufs=3, name="scbi")
            h2S = st["h2S"]
            nc.tensor.matmul(scbi[:, 0:512], t_sw[L], h2S[:, 0:512],
                             start=True, stop=False)
            nc.tensor.matmul(scbi[:, 512:PAIR], t_sw[L], h2S[:, 512:PAIR],
                             start=True, stop=False)
            gS = ewp.tile([C, PAIR], BF16, tag="gS", bufs=6, name="gS")
            nc.vector.tensor_tensor(gS[:], st["h2S"][:], dN[:], op=ALU.mult)
            st["gS"] = gS
            if L == 0 and not emit_trunk_bias:
                yc_in = st.pop("xB")[:]
            else:
                yc_in = st.pop("ycS")[:]
            nc.vector.scalar_tensor_tensor(
                scbi[:], scbi[:], t_sb[L], yc_in,
                op0=ALU.add, op1=ALU.mult)
            st["scbi"] = scbi

        def lay_r6(p, L):
            st = state[p]
            gS = st.pop("gS")
            dN = st["dN"]
            scbi = st["scbi"]
            stp = not emit_style_bias
            nc.tensor.matmul(scbi[:, 0:512], t_bw[L], gS[:, 0:512],
                             start=False, stop=stp, skip_group_check=True)
            nc.tensor.matmul(scbi[:, 512:PAIR], t_bw[L], gS[:, 512:PAIR],
                             start=False, stop=stp, skip_group_check=True)
            if emit_style_bias:
                nc.tensor.matmul(scbi[:, 0:512], t_bbrow[L], dN[0:1, 0:512],
                                 start=False, stop=True,
                                 skip_group_check=True)
                nc.tensor.matmul(scbi[:, 512:PAIR], t_bbrow[L],
                                 dN[0:1, 512:PAIR], start=False, stop=True,
                                 skip_group_check=True)

        def lay_r7(p, L):
            st = state[p]
            zS = ewp.tile([C, PAIR], F32R, tag="zS", bufs=5, name="zS")
            nc.scalar.activation(zS[:], st.pop("scbi")[:], AF.Prelu,
                                 alpha=0.01)
            st["zS"] = zS
            if L < 3:
                st["dS"] = st.pop("dN")
            else:
                st["dS"] = st["dN"]

        def out_r0(p):
            st = state[p]
            zS, dSp = st.pop("zS"), st["dS"]
            outP = psp.tile([1, PAIR], F32, tag="A", bufs=3, name="outP")
            stp = b5_val == 0.0
            nc.tensor.matmul(outP[0:1, 0:512], t_w5, zS[:, 0:512],
                             start=True, stop=stp)
            nc.tensor.matmul(outP[0:1, 512:PAIR], t_w5, zS[:, 512:PAIR],
                             start=True, stop=stp)
            if not stp:
                nc.tensor.matmul(outP[0:1, 0:512], t_b5, dSp[0:1, 0:512],
                                 start=False, stop=True)
                nc.tensor.matmul(outP[0:1, 512:PAIR], t_b5,
                                 dSp[0:1, 512:PAIR],
                                 start=False, stop=True)
            st["outP"] = outP

        def out_r1(p):
            st = state[p]
            oS = ewp.tile([1, PAIR], F32, tag="oS", bufs=2, name="oS")
            nc.scalar.activation(oS[:], st.pop("outP")[:], AF.Prelu,
                                 alpha=0.01)
            st["oS"] = oS

        def out_r7(p):
            st = state[p]
            nc.sync.dma_start(d_out[2 * p:2 * p + 2, :],
                              st.pop("oS")[0:1, :])
            nc.sync.dma_start(d_den[2 * p:2 * p + 2, :],
                              st["dS"][0:1, :])
            state[p] = {}

        def make_subs(p):
            subs = [lambda: pre_r0(p),
                    lambda: head_r0(p), lambda: head_r1(p),
                    lambda: head_r2(p), lambda: head_r3(p)]
            for L in range(4):
                for fn in (lay_r0, lay_r1, lay_r2, lay_r3, lay_r4, lay_r5,
                           lay_r6, lay_r7):
                    subs.append(lambda fn=fn, L=L: fn(p, L))
            subs += [lambda: out_r0(p), lambda: out_r1(p),
                     lambda: out_r7(p)]
            return subs

        SUBS = [make_subs(p) for p in range(NPAIR)]
        NSUB = len(SUBS[0])
        STAG = 6
        for tau in range(NSUB + STAG * (NPAIR - 1)):
            for p in range(NPAIR):
                i = tau - STAG * p
                if 0 <= i < NSUB:
                    SUBS[p][i]()

    nc.compile()
    return nc


def _prep(x, metadata, mw1, mb1, mw2, mb2, mw3, mb3,
          w1, b1, w2, b2, w3, b3, w4, b4, w5, b5):
    f = np.float32
    even = 2 * np.arange(C)
    mw3 = np.asarray(mw3, f)
    mb3 = np.asarray(mb3, f)

    w1c = np.asarray(w1, f).reshape(C) - float(np.asarray(w1, f).mean())
    k = float(np.sqrt((w1c ** 2).sum() / (C - 1)))
    sgn = np.where(w1c == 0.0, 1.0, np.sign(w1c)).astype(f)
    w1pp = (sgn * k).astype(f)                    # w1'' row
    bct = [np.asarray(b, f) - float(np.asarray(b, f).mean())
           for b in (b1, b2, b3, b4)]
    emit_tb = bool(any(np.any(b) for b in bct))
    l0_fast = (not emit_tb) and not os.environ.get("L0ABS")
    if l0_fast:
        # fast path: L0 yc = w1''*x folds into the style scale entirely
        # (l0scale * w1'' == w1c); m1_0 = (SW0'' h2 + sb0'') .* broadcast(x)
        l0scale = w1c
    else:
        l0scale = (np.abs(w1c) / k).astype(f)     # per-channel SW_L0 scale

    def center(w):
        w = np.asarray(w, f)
        return (w - w.mean(axis=0, keepdims=True)).astype(f)

    wpack = np.zeros((C, NW), f)
    wpack16 = np.zeros((C, NW16), f)
    # packed mw1 block-diag [32, 128]
    m1t = np.asarray(mw1, f).T                    # [16, 64]
    wpack[0:MF, _O_MW1:_O_MW1 + 64] = m1t
    wpack[MF:2 * MF, _O_MW1 + 64:_O_MW1 + 128] = m1t
    # mw2 + duplicate
    m2t = np.asarray(mw2, f).T                    # [64, 128]
    wpack[0:64, _O_MW2:_O_MW2 + 128] = m2t
    wpack[64:128, _O_MW2:_O_MW2 + 128] = m2t
    # style weights
    sb = np.zeros((C, 4), f)
    bbrow = np.zeros((4, C), f)
    for L in range(4):
        rows = 256 * L + even
        swt = mw3[rows, :].T.copy()               # [128 in, 128 out]
        sbL = mb3[rows].copy()
        if L == 0:
            swt *= l0scale[None, :]
            sbL = sbL * l0scale
        wpack16[:, _H_SW + L * C:_H_SW + (L + 1) * C] = swt
        wpack16[:, _H_BW + L * C:_H_BW + (L + 1) * C] = mw3[rows + 1, :].T
        sb[:, L] = sbL
        bbrow[L] = mb3[rows + 1]
    # trunk
    for i, w in enumerate((w2, w3, w4)):
        wpack[:, _O_WC + i * C:_O_WC + (i + 1) * C] = center(w).T
    wpack16[:, _H_ONES:_H_ONES + C] = 1.0
    wpack[0:1, _O_W1:_O_W1 + C] = w1pp[None, :]
    wpack[:, _O_W5:_O_W5 + 1] = np.asarray(w5, f).reshape(1, C).T
    for L in range(4):
        wpack16[0:1, _H_BROW + L * C:_H_BROW + (L + 1) * C] = bct[L][None, :]
        wpack16[0:1, _H_BBROW + L * C:_H_BBROW + (L + 1) * C] = \
            bbrow[L][None, :]
    wpack16[0, _H_B5] = float(np.asarray(b5).reshape(-1)[0])

    bpack = np.zeros((C, NBP), f)
    mb1v = np.asarray(mb1, f)
    bpack[0:64, _BP_MB1] = mb1v
    bpack[64:128, _BP_MB1] = mb1v
    bpack[:, _BP_MB2] = np.asarray(mb2, f)
    bpack[:, _BP_SB0:_BP_SB0 + 4] = sb
    bpack[:, _BP_B1C] = bct[0]

    emit_sb = bool(np.any(bbrow))
    emit_tb = emit_tb or not l0_fast
    b5v = float(np.asarray(b5).reshape(-1)[0])

    import ml_dtypes
    xv = np.asarray(x, f).reshape(B)
    xv16 = xv.astype(ml_dtypes.bfloat16)
    # d_0 = k|x| from the SAME bf16 x the kernel multiplies with, so the
    # x-quantization cancels in out/den
    xav16 = (np.float32(k) * np.abs(xv16.astype(f))).astype(ml_dtypes.bfloat16)
    xav = None
    wpack16 = wpack16.astype(ml_dtypes.bfloat16)
    mdv = np.asarray(metadata, f)
    shared = dict(wpack=wpack, wpack16=wpack16, bpack=bpack)
    in_maps = []
    for c in range(NCORES):
        m = dict(shared)
        xs = xv[c * BL:(c + 1) * BL]
        m["xt"] = np.ascontiguousarray(xs.reshape(1, BL))
        m["xat"] = np.ascontiguousarray(
            xav16[c * BL:(c + 1) * BL].reshape(1, BL))
        m["xt16"] = np.ascontiguousarray(
            xv16[c * BL:(c + 1) * BL].reshape(1, BL))
        md = mdv[c * BL:(c + 1) * BL, :]          # [BL, 16]
        # pair-stacked layout [32, BL/2]: rows 0:16 = first 512 of each pair,
        # rows 16:32 = second 512 of each pair
        md4 = md.reshape(NPAIR, 2, 512, MF)       # [pair, half, col, feat]
        mdt = md4.transpose(1, 3, 0, 2).reshape(2 * MF, NPAIR * 512)
        m["mdt"] = np.ascontiguousarray(mdt)
        in_maps.append(m)
    return in_maps, emit_sb, emit_tb, b5v


def run(trace=False, reps=1, **inputs):
    in_maps, esb, etb, b5v = _prep(**inputs)
    nc = _build(esb, etb, b5v, reps=reps)
    res = bass_utils.run_bass_kernel_spmd(
        nc, in_maps, core_ids=list(range(NCORES)), trace=trace)
    out = np.concatenate([
        (np.asarray(res.results[c]["out"]).reshape(BL).astype(np.float32)
         / np.asarray(res.results[c]["den"]).reshape(BL).astype(np.float32))
        for c in range(NCORES)
    ]).reshape(B, 1).astype(np.float32)
    return out, res


def kernel(**inputs):
    out, _ = run(trace=False, **inputs)
    return out


# revision 27
# speedup vs baseline: 1.3700x; 1.3700x over previous
"""Trainium2 Bass kernel v7 for nn_AttenuationToRainRate (dense_mlp).

Data-parallel over 8 NeuronCores, BL = 32768 samples/core, processed in
32 "pairs" of 1024 samples (2 PSUM banks wide).

Math per sample (channels c on partitions, samples along free dim):
  style: h2 = relu(mw2 relu(mw1 md + mb1) + mb2)
  layer L: u = (SW_L h2 + sb_L) .* yc_L  +  (BW_L (h2 .* d_L) + bb_L*d_L)
           z_L = lrelu(u)            [scaled-denominator trick: no division]
  d_L = sqrt(sum_c yc_L^2 / 127) broadcast via ones-matmul (L>=1)
  L0 fold: yc_0 = w1''*x is rank-1, so w1c folds into SW_0 on the host and
           m1_0 = (SW0'' h2 + sb0'') .* broadcast(x); d_0 = k|x| row.
           Both broadcasts are stride-0-partition DMA reads from DRAM.
  out = lrelu(w5 z_3 + b5*d_3) / d_3   (division on host)

v7 engine assignment per pair (1024 cols):
  PE   : 33 matmuls (512-col each); SW/BW/ones in bf16, trunk in f32r
  Act  : h1/h2 relu, ycS sink x3, d sqrt x3, out prelu          (9 ops)
  DVE  : m1 = (scP+sb).*ycS (stt),  z = lrelu(m1+biP) (custom)  (8 ops)
  Pool : sq = ycS^2 x3, g = h2.*d x4 -- all-bf16 (gpsimd is sw-
         throughput-bound; bf16 halves its byte traffic)        (7 ops)
  DMA  : xB/d0 row broadcasts (DRAM stride-0 partition APs)
"""

import os
import sys

import numpy as np

for p in ("/opt/trn_rl_repo", "/root/.axon_site/_ro/trn_rl_repo"):
    if os.path.isdir(p) and p not in sys.path:
        sys.path.insert(0, p)


import concourse.bass as bass
import concourse.bacc as bacc
import concourse.mybir as mybir
from concourse.bass import AP
from concourse.tile import TileContext
from concourse import bass_utils

# ---- inline custom DVE op registration (kernel.py must be self-contained) --
import concourse.dve_ops as dve_ops
from concourse.dve_ops import DveOp
from concourse.dve_spec import Spec, Src0, Src1, C0, maxx, lower, _has_src1
from concourse.dve_uop import DveOpSpec
from concourse.dve_table_gen import dve_ver_for


def _register(name, spec):
    if name in dve_ops._SUB_OPCODE_FOR_NAME:
        for op in dve_ops.OPS:
            if op.name == name:
                return op
    row = max(dve_ops._SUB_OPCODE_FOR_NAME.values()) + 1
    assert row < 0x20, "custom DVE opcode row overflow"
    dve_ops._SUB_OPCODE_FOR_NAME[name] = row
    shas = {}
    for ver in ("v3", "v4"):
        try:
            tmp = DveOpSpec(name=name, opcode=row, uops=lower(spec, ver=ver),
                            rd1_en=_has_src1(spec))
            shas[ver] = tmp.sha(ver)
        except Exception:
            pass
    op = DveOp(name, spec, subdim=False, uops_sha=shas)
    dve_ops.OPS.append(op)
    dve_ops.CUSTOM_DVE_SPECS[name] = spec
    return op


def _lrelu_add_spec():
    u = Src0 + Src1
    return Spec(
        body=maxx(u, u * C0),
        reference=lambda in0, in1, s0, s1: np.maximum(in0 + in1,
                                                      (in0 + in1) * s0),
    )


LRELU_ADD_ANT = _register("LRELU_ADD_ANT", _lrelu_add_spec())


class custom_ops:
    LRELU_ADD_ANT = LRELU_ADD_ANT

B = 262144
MF = 16
C = 128
NCORES = 8
BL = B // NCORES            # 32768 samples per core
PAIR = 1024                 # samples per pair (2 PSUM banks)
NPAIR = BL // PAIR          # 32
F32R = mybir.dt.float32r
F32 = mybir.dt.float32
BF16 = mybir.dt.bfloat16
AF = mybir.ActivationFunctionType
ALU = mybir.AluOpType

# wpack column layout (f32r part)
_O_MW1 = 0        # [32 rows, 128] packed block-diag mw1 (2 chunks)
_O_MW2 = 128      # [0:64 rows, 128] mw2 ; [64:128 rows] duplicate
_O_WC = 256       # 3 x [128,128] centered trunk w2..w4
_O_W1 = 640       # [1 row, 128] w1'' = sign(w1c)*k (general path only)
_O_W5 = 768       # [128, 1] w5
NW = 769

# wpack16 column layout (bf16 part)
_H_SW = 0         # 4 x [128,128] scale style weights (L0 folded w/ w1c)
_H_BW = 512       # 4 x [128,128] bias style weights
_H_ONES = 1024    # [128,128] ones
_H_BROW = 1152    # [1 row, 4*128] bct rows (trunk bias, emit_tb)
_H_BBROW = 1664   # [1 row, 4*128] bb rows (style bias, emit_sb)
_H_B5 = 2176      # [1 row, 1] b5
NW16 = 2177

# bpack [128, 8] f32: cols = mb1packed, mb2, sb0..sb3, b1c
_BP_MB1, _BP_MB2, _BP_SB0, _BP_B1C = 0, 1, 2, 6
NBP = 7


def _build(emit_style_bias, emit_trunk_bias, b5_val, reps=1):
    nc = bacc.Bacc("TRN2", target_bir_lowering=False, debug=False)

    d_x = nc.dram_tensor("xt", [1, BL], F32R, kind="ExternalInput")
    d_md = nc.dram_tensor("mdt", [32, BL // 2], F32R, kind="ExternalInput")
    d_wp = nc.dram_tensor("wpack", [C, NW], F32R, kind="ExternalInput")
    d_wp16 = nc.dram_tensor("wpack16", [C, NW16], BF16, kind="ExternalInput")
    d_bp = nc.dram_tensor("bpack", [C, NBP], F32, kind="ExternalInput")
    d_xa = nc.dram_tensor("xat", [1, BL], BF16, kind="ExternalInput")
    d_x16 = nc.dram_tensor("xt16", [1, BL], BF16, kind="ExternalInput")
    d_out = nc.dram_tensor("out", [BL // 512, 512], F32, kind="ExternalOutput")
    d_den = nc.dram_tensor("den", [BL // 512, 512], BF16,
                           kind="ExternalOutput")

    from contextlib import ExitStack
    with TileContext(nc) as tc, ExitStack() as es:
        wp = es.enter_context(tc.tile_pool(name="wp", bufs=1))
        iop = es.enter_context(tc.tile_pool(name="iop", bufs=2))
        ewp = es.enter_context(tc.tile_pool(name="ewp", bufs=2))
        psp = es.enter_context(tc.tile_pool(name="psp", bufs=1, space="PSUM"))

        t_wp = wp.tile([C, NW], F32R)
        nc.sync.dma_start(t_wp[:], d_wp[:])
        t_wp16 = wp.tile([C, NW16], BF16)
        nc.sync.dma_start(t_wp16[:], d_wp16[:])
        t_bp = wp.tile([C, NBP], F32)
        nc.sync.dma_start(t_bp[:], d_bp[:])

        t_mw1 = t_wp[0:32, _O_MW1:_O_MW1 + 128]
        t_mw2a = t_wp[0:64, _O_MW2:_O_MW2 + 128]
        t_mw2b = t_wp[64:128, _O_MW2:_O_MW2 + 128]
        t_wc = [t_wp[:, _O_WC + i * C:_O_WC + (i + 1) * C] for i in range(3)]
        t_w1 = t_wp[0:1, _O_W1:_O_W1 + C]
        t_w5 = t_wp[:, _O_W5:_O_W5 + 1]
        t_sw = [t_wp16[:, _H_SW + L * C:_H_SW + (L + 1) * C]
                for L in range(4)]
        t_bw = [t_wp16[:, _H_BW + L * C:_H_BW + (L + 1) * C]
                for L in range(4)]
        t_ones = t_wp16[:, _H_ONES:_H_ONES + C]
        t_brow = [t_wp16[0:1, _H_BROW + L * C:_H_BROW + (L + 1) * C]
                  for L in range(4)]
        t_bbrow = [t_wp16[0:1, _H_BBROW + L * C:_H_BBROW + (L + 1) * C]
                   for L in range(4)]
        t_b5 = t_wp16[0:1, _H_B5:_H_B5 + 1]
        t_mb1 = t_bp[:, _BP_MB1:_BP_MB1 + 1]
        t_mb2 = t_bp[:, _BP_MB2:_BP_MB2 + 1]
        t_sb = [t_bp[:, _BP_SB0 + L:_BP_SB0 + L + 1] for L in range(4)]
        t_b1c = t_bp[:, _BP_B1C:_BP_B1C + 1]

        rep_cm = tc.For_i(0, reps, 1) if reps > 1 else None
        if rep_cm is not None:
            es.enter_context(rep_cm)

        state = [dict() for _ in range(NPAIR)]

        def _dram_bcast(dst_ap, dram_slice):
            # partition-broadcast from DRAM: re-read the same row into all
            # 128 partitions (stride-0 partition dim is legal on the DRAM
            # side only)
            src = AP(dram_slice.tensor, dram_slice.offset,
                     [[0, C], [1, PAIR]])
            nc.sync.dma_start(dst_ap, src)

        def pre_r0(p):
            st = state[p]
            t_md = iop.tile([32, 512], F32R, tag="md", bufs=4, name="t_md")
            nc.sync.dma_start(t_md[:], d_md[:, p * 512:(p + 1) * 512])
            st["md"] = t_md
            if emit_trunk_bias:
                t_x = iop.tile([1, PAIR], F32R, tag="x", bufs=2, name="t_x")
                nc.sync.dma_start(t_x[:], d_x[:, p * PAIR:(p + 1) * PAIR])
                st["x"] = t_x

        def head_r0(p):
            st = state[p]
            h1P = psp.tile([C, 512], F32, tag="S", bufs=2, name="h1P")
            nc.tensor.matmul(h1P[:], t_mw1, st.pop("md")[:],
                             start=True, stop=True)
            st["h1P"] = h1P
            if not emit_trunk_bias:
                # broadcast x and d0 = k|x| rows while the head computes
                xB = ewp.tile([C, PAIR], BF16, tag="xB", bufs=4, name="xB")
                _dram_bcast(xB[:], d_x16[0:1, p * PAIR:(p + 1) * PAIR])
                st["xB"] = xB
                dN = ewp.tile([C, PAIR], BF16, tag="dS", bufs=5, name="dN0")
                _dram_bcast(dN[:], d_xa[0:1, p * PAIR:(p + 1) * PAIR])
                st["dN"] = dN

        def head_r1(p):
            st = state[p]
            h1S = ewp.tile([C, 512], F32R, tag="h1S", bufs=2, name="h1S")
            nc.scalar.activation(h1S[:], st.pop("h1P")[:], AF.Relu,
                                 bias=t_mb1)
            st["h1S"] = h1S

        def head_r2(p):
            st = state[p]
            h1S = st.pop("h1S")
            h2P = psp.tile([C, PAIR], F32, tag="A", bufs=3, name="h2P")
            nc.tensor.matmul(h2P[:, 0:512], t_mw2a, h1S[0:64, :],
                             start=True, stop=True)
            nc.tensor.matmul(h2P[:, 512:PAIR], t_mw2b, h1S[64:128, :],
                             start=True, stop=True)
            st["h2P"] = h2P

        def head_r3(p):
            st = state[p]
            h2S = ewp.tile([C, PAIR], BF16, tag="h2S", bufs=6, name="h2S")
            nc.scalar.activation(h2S[:], st.pop("h2P")[:], AF.Relu,
                                 bias=t_mb2)
            st["h2S"] = h2S

        def lay_r0(p, L):
            st = state[p]
            if L == 0:
                if not emit_trunk_bias:
                    return
                ycP = psp.tile([C, PAIR], F32, tag="A", bufs=3, name="ycP")
                t_x = st.pop("x")
                nc.tensor.matmul(ycP[:, 0:512], t_w1, t_x[:, 0:512],
                                 start=True, stop=True)
                nc.tensor.matmul(ycP[:, 512:PAIR], t_w1, t_x[:, 512:PAIR],
                                 start=True, stop=True)
                st["ycP"] = ycP
                return
            ycP = psp.tile([C, PAIR], F32, tag="A", bufs=3, name="ycP")
            zS, dSp = st.pop("zS"), st["dS"]
            stp = not emit_trunk_bias
            nc.tensor.matmul(ycP[:, 0:512], t_wc[L - 1], zS[:, 0:512],
                             start=True, stop=stp)
            nc.tensor.matmul(ycP[:, 512:PAIR], t_wc[L - 1],
                             zS[:, 512:PAIR], start=True, stop=stp)
            if emit_trunk_bias:
                nc.tensor.matmul(ycP[:, 0:512], t_brow[L],
                                 dSp[0:1, 0:512], start=False, stop=True)
                nc.tensor.matmul(ycP[:, 512:PAIR], t_brow[L],
                                 dSp[0:1, 512:PAIR],
                                 start=False, stop=True)
            st["ycP"] = ycP

        def lay_r1(p, L):
            st = state[p]
            if L == 0 and not emit_trunk_bias:
                return
            # fast PSUM sink: ycS frees ycP's banks quickly (PSUM rotation
            # is the scarce resource); squares then run off SBUF on Pool.
            # L1 sinks on Act, L2/L3 on DVE (engine balance).
            ycS = ewp.tile([C, PAIR], BF16, tag="ycS", bufs=5, name="ycS")
            if L == 0 and emit_trunk_bias:
                nc.scalar.activation(ycS[:], st.pop("ycP")[:], AF.Identity,
                                     bias=t_b1c)
            elif L >= 2:
                nc.vector.tensor_copy(ycS[:], st.pop("ycP")[:])
            else:
                nc.scalar.activation(ycS[:], st.pop("ycP")[:], AF.Identity)
            st["ycS"] = ycS

        def lay_r2(p, L):
            st = state[p]
            if L == 0:
                if emit_trunk_bias:
                    dN = ewp.tile([C, PAIR], BF16, tag="dS", bufs=5,
                                  name="dN")
                    nc.scalar.activation(dN[:], st["ycS"][:], AF.Abs)
                    st["dN"] = dN
            else:
                ycS = st["ycS"]
                sqS = ewp.tile([C, PAIR], BF16, tag="sq", bufs=5, name="sqS")
                nc.gpsimd.tensor_mul(sqS[:], ycS[:], ycS[:])
                st["sqS"] = sqS

        def lay_r3(p, L):
            st = state[p]
            if L > 0:
                sqS = st.pop("sqS")
                vP = psp.tile([C, PAIR], F32, tag="A", bufs=3, name="vP")
                nc.tensor.matmul(vP[:, 0:512], t_ones, sqS[:, 0:512],
                                 start=True, stop=True)
                nc.tensor.matmul(vP[:, 512:PAIR], t_ones, sqS[:, 512:PAIR],
                                 start=True, stop=True)
                st["vP"] = vP

        def lay_r4(p, L):
            st = state[p]
            if L > 0:
                dN = ewp.tile([C, PAIR], BF16, tag="dS", bufs=5, name="dN")
                nc.scalar.activation(dN[:], st.pop("vP")[:], AF.Sqrt,
                                     scale=1.0 / (C - 1))
                st["dN"] = dN
            # scbi: merged scale/bias PSUM bank. scP matmul opens the
            # accumulation (start, no stop); the DVE m1 RMW rewrites the
            # values (has_written bits survive); the BW matmul then
            # accumulates the bias path on top.
            scbi = psp.tile([C, PAIR], F32, tag="A", bufs=3, name="scbi")
            h2S = st["h2S"]
            nc.tensor.matmul(scbi[:, 0:512], t_sw[L], h2S[:, 0:512],
                             start=True, stop=False)
            nc.tensor.matmul(scbi[:, 512:PAIR], t_sw[L], h2S[:, 512:PAIR],
                             start=True, stop=False)
            st["scbi"] = scbi

        def lay_r5(p, L):
            st = state[p]
            dN = st["dN"]
            gS = ewp.tile([C, PAIR], BF16, tag="gS", bufs=6, name="gS")
            nc.vector.tensor_tensor(gS[:], st["h2S"][:], dN[:], op=ALU.mult)
            st["gS"] = gS
            scbi = st["scbi"]
            if L == 0 and not emit_trunk_bias:
                yc_in = st.pop("xB")[:]
            else:
                yc_in = st.pop("ycS")[:]
            nc.vector.scalar_tensor_tensor(
                scbi[:], scbi[:], t_sb[L], yc_in,
                op0=ALU.add, op1=ALU.mult)

        def lay_r6(p, L):
            st = state[p]
            gS = st.pop("gS")
            dN = st["dN"]
            scbi = st["scbi"]
            stp = not emit_style_bias
            nc.tensor.matmul(scbi[:, 0:512], t_bw[L], gS[:, 0:512],
                             start=False, stop=stp, skip_group_check=True)
            nc.tensor.matmul(scbi[:, 512:PAIR], t_bw[L], gS[:, 512:PAIR],
                             start=False, stop=stp, skip_group_check=True)
            if emit_style_bias:
                nc.tensor.matmul(scbi[:, 0:512], t_bbrow[L], dN[0:1, 0:512],
                                 start=False, stop=True,
                                 skip_group_check=True)
                nc.tensor.matmul(scbi[:, 512:PAIR], t_bbrow[L],
                                 dN[0:1, 512:PAIR], start=False, stop=True,
                                 skip_group_check=True)

        def lay_r7(p, L):
            st = state[p]
            zS = ewp.tile([C, PAIR], F32R, tag="zS", bufs=5, name="zS")
            nc.scalar.activation(zS[:], st.pop("scbi")[:], AF.Prelu,
                                 alpha=0.01)
            st["zS"] = zS
            if L < 3:
                st["dS"] = st.pop("dN")
            else:
                st["dS"] = st["dN"]

        def out_r0(p):
            st = state[p]
            zS, dSp = st.pop("zS"), st["dS"]
            outP = psp.tile([1, PAIR], F32, tag="A", bufs=3, name="outP")
            stp = b5_val == 0.0
            nc.tensor.matmul(outP[0:1, 0:512], t_w5, zS[:, 0:512],
                             start=True, stop=stp)
            nc.tensor.matmul(outP[0:1, 512:PAIR], t_w5, zS[:, 512:PAIR],
                             start=True, stop=stp)
            if not stp:
                nc.tensor.matmul(outP[0:1, 0:512], t_b5, dSp[0:1, 0:512],
                                 start=False, stop=True)
                nc.tensor.matmul(outP[0:1, 512:PAIR], t_b5,
                                 dSp[0:1, 512:PAIR],
                                 start=False, stop=True)
            st["outP"] = outP

        def out_r1(p):
            st = state[p]
            oS = ewp.tile([1, PAIR], F32, tag="oS", bufs=2, name="oS")
            nc.scalar.activation(oS[:], st.pop("outP")[:], AF.Prelu,
                                 alpha=0.01)
            st["oS"] = oS

        def out_r7(p):
            st = state[p]
            nc.sync.dma_start(d_out[2 * p:2 * p + 2, :],
                              st.pop("oS")[0:1, :])
            nc.sync.dma_start(d_den[2 * p:2 * p + 2, :],
                              st["dS"][0:1, :])
            state[p] = {}

        def make_subs(p):
            subs = [lambda: pre_r0(p),
                    lambda: head_r0(p), lambda: head_r1(p),
                    lambda: head_r2(p), lambda: head_r3(p)]
            for L in range(4):
                for fn in (lay_r0, lay_r1, lay_r2, lay_r3, lay_r4, lay_r5,
                           lay_r6, lay_r7):
                    subs.append(lambda fn=fn, L=L: fn(p, L))
            subs += [lambda: out_r0(p), lambda: out_r1(p),
                     lambda: out_r7(p)]
            return subs

        SUBS = [make_subs(p) for p in range(NPAIR)]
        NSUB = len(SUBS[0])
        STAG = 6
        for tau in range(NSUB + STAG * (NPAIR - 1)):
            for p in range(NPAIR):
                i = tau - STAG * p
                if 0 <= i < NSUB:
                    SUBS[p][i]()

    nc.compile()
    return nc


def _prep(x, metadata, mw1, mb1, mw2, mb2, mw3, mb3,
          w1, b1, w2, b2, w3, b3, w4, b4, w5, b5):
    f = np.float32
    even = 2 * np.arange(C)
    mw3 = np.asarray(mw3, f)
    mb3 = np.asarray(mb3, f)

    w1c = np.asarray(w1, f).reshape(C) - float(np.asarray(w1, f).mean())
    k = float(np.sqrt((w1c ** 2).sum() / (C - 1)))
    sgn = np.where(w1c == 0.0, 1.0, np.sign(w1c)).astype(f)
    w1pp = (sgn * k).astype(f)                    # w1'' row
    bct = [np.asarray(b, f) - float(np.asarray(b, f).mean())
           for b in (b1, b2, b3, b4)]
    emit_tb = bool(any(np.any(b) for b in bct))
    l0_fast = (not emit_tb) and not os.environ.get("L0ABS")
    if l0_fast:
        # fast path: L0 yc = w1''*x folds into the style scale entirely
        # (l0scale * w1'' == w1c); m1_0 = (SW0'' h2 + sb0'') .* broadcast(x)
        l0scale = w1c
    else:
        l0scale = (np.abs(w1c) / k).astype(f)     # per-channel SW_L0 scale

    def center(w):
        w = np.asarray(w, f)
        return (w - w.mean(axis=0, keepdims=True)).astype(f)

    wpack = np.zeros((C, NW), f)
    wpack16 = np.zeros((C, NW16), f)
    # packed mw1 block-diag [32, 128]
    m1t = np.asarray(mw1, f).T                    # [16, 64]
    wpack[0:MF, _O_MW1:_O_MW1 + 64] = m1t
    wpack[MF:2 * MF, _O_MW1 + 64:_O_MW1 + 128] = m1t
    # mw2 + duplicate
    m2t = np.asarray(mw2, f).T                    # [64, 128]
    wpack[0:64, _O_MW2:_O_MW2 + 128] = m2t
    wpack[64:128, _O_MW2:_O_MW2 + 128] = m2t
    # style weights
    sb = np.zeros((C, 4), f)
    bbrow = np.zeros((4, C), f)
    for L in range(4):
        rows = 256 * L + even
        swt = mw3[rows, :].T.copy()               # [128 in, 128 out]
        sbL = mb3[rows].copy()
        if L == 0:
            swt *= l0scale[None, :]
            sbL = sbL * l0scale
        wpack16[:, _H_SW + L * C:_H_SW + (L + 1) * C] = swt
        wpack16[:, _H_BW + L * C:_H_BW + (L + 1) * C] = mw3[rows + 1, :].T
        sb[:, L] = sbL
        bbrow[L] = mb3[rows + 1]
    # trunk
    for i, w in enumerate((w2, w3, w4)):
        wpack[:, _O_WC + i * C:_O_WC + (i + 1) * C] = center(w).T
    wpack16[:, _H_ONES:_H_ONES + C] = 1.0
    wpack[0:1, _O_W1:_O_W1 + C] = w1pp[None, :]
    wpack[:, _O_W5:_O_W5 + 1] = np.asarray(w5, f).reshape(1, C).T
    for L in range(4):
        wpack16[0:1, _H_BROW + L * C:_H_BROW + (L + 1) * C] = bct[L][None, :]
        wpack16[0:1, _H_BBROW + L * C:_H_BBROW + (L + 1) * C] = \
            bbrow[L][None, :]
    wpack16[0, _H_B5] = float(np.asarray(b5).reshape(-1)[0])

    bpack = np.zeros((C, NBP), f)
    mb1v = np.asarray(mb1, f)
    bpack[0:64, _BP_MB1] = mb1v
    bpack[64:128, _BP_MB1] = mb1v
    bpack[:, _BP_MB2] = np.asarray(mb2, f)
    bpack[:, _BP_SB0:_BP_SB0 + 4] = sb
    bpack[:, _BP_B1C] = bct[0]

    emit_sb = bool(np.any(bbrow))
    emit_tb = emit_tb or not l0_fast
    b5v = float(np.asarray(b5).reshape(-1)[0])

    import ml_dtypes
    xv = np.asarray(x, f).reshape(B)
    xv16 = xv.astype(ml_dtypes.bfloat16)
    # d_0 = k|x| from the SAME bf16 x the kernel multiplies with, so the
    # x-quantization cancels in out/den
    xav16 = (np.float32(k) * np.abs(xv16.astype(f))).astype(ml_dtypes.bfloat16)
    xav = None
    wpack16 = wpack16.astype(ml_dtypes.bfloat16)
    mdv = np.asarray(metadata, f)
    shared = dict(wpack=wpack, wpack16=wpack16, bpack=bpack)
    in_maps = []
    for c in range(NCORES):
        m = dict(shared)
        xs = xv[c * BL:(c + 1) * BL]
        m["xt"] = np.ascontiguousarray(xs.reshape(1, BL))
        m["xat"] = np.ascontiguousarray(
            xav16[c * BL:(c + 1) * BL].reshape(1, BL))
        m["xt16"] = np.ascontiguousarray(
            xv16[c * BL:(c + 1) * BL].reshape(1, BL))
        md = mdv[c * BL:(c + 1) * BL, :]          # [BL, 16]
        # pair-stacked layout [32, BL/2]: rows 0:16 = first 512 of each pair,
        # rows 16:32 = second 512 of each pair
        md4 = md.reshape(NPAIR, 2, 512, MF)       # [pair, half, col, feat]
        mdt = md4.transpose(1, 3, 0, 2).reshape(2 * MF, NPAIR * 512)
        m["mdt"] = np.ascontiguousarray(mdt)
        in_maps.append(m)
    return in_maps, emit_sb, emit_tb, b5v


def run(trace=False, reps=1, **inputs):
    in_maps, esb, etb, b5v = _prep(**inputs)
    nc = _build(esb, etb, b5v, reps=reps)
    res = bass_utils.run_bass_kernel_spmd(
        nc, in_maps, core_ids=list(range(NCORES)), trace=trace)
    out = np.concatenate([
        (np.asarray(res.results[c]["out"]).reshape(BL).astype(np.float32)
         / np.asarray(res.results[c]["den"]).reshape(BL).astype(np.float32))
        for c in range(NCORES)
    ]).reshape(B, 1).astype(np.float32)
    return out, res


def kernel(**inputs):
    out, _ = run(trace=False, **inputs)
    return out
